# revision 1
# baseline (speedup 1.0000x reference)
"""Bass kernel for nn_Attn_1898375545663 on 8 TRN2 NeuronCores.

Reference (single device):
    energies[b, l] = sum_h hidden[h, b] * encoder_outputs[l, b, h]   # [B, L]
    attn = softmax(energies, axis=1)                                 # [B, L]
    return attn[:, None, :]                                          # [B, 1, L]

Shapes: L=4096, B=32, H=1024, fp32. encoder_outputs is 512 MB -> memory bound.

Sharding: pure data parallel over batch. Each of the 8 cores gets 4 batches
(encoder shard [4096, 4, 1024] = 64 MB); no collectives.

Per-core kernel:
  - hidden shard arrives as one row [1, 4*1024]; gpsimd.partition_broadcast
    replicates it to [128, 4096] so the DVE can use it per-partition.
  - Main loop: DMA encoder tiles [128 l-rows, TG tiles, 1024 h] (2 MB per
    dma_start for DMA efficiency), then one fused DVE affine_mul_reduce per
    (batch, l-tile): the elementwise product goes to a stride-0 dummy, the
    free-axis (h) sum lands in en[:, c] (c = b*32 + t). One DVE pass/element.
  - Softmax: global max over all 4 batches (any per-batch constant is exact
    for softmax; the global max keeps exp in range), exp on ScalarE,
    per-batch sums via a PE matmul against a ones vector, reciprocal, then a
    PE transpose so each (b, t) partition holds 128 contiguous l values,
    scale by 1/sum, one output DMA.
"""

import numpy as np

from concourse import bacc, mybir, tile
from concourse.bass_isa import ReduceOp
from concourse.bass_utils import run_bass_kernel_spmd
from concourse.masks import make_identity

L, B, H = 4096, 32, 1024
NCORES = 8
BS = B // NCORES          # 4 batches per core
P = 128                   # partitions / l-tile height
NT = L // P               # 32 l-tiles per batch
TG = 4                    # l-tiles per DMA group (2 MB per dma_start)
NC_COLS = BS * NT         # 128 energy columns per core
F32 = mybir.dt.float32

_cached = {}


def main_loop(nc, inp, enc, hidb, en, dummy, order="b"):
    pairs = (
        [(b, g) for b in range(BS) for g in range(NT // TG)]
        if order == "b"
        else [(b, g) for g in range(NT // TG) for b in range(BS)]
    )
    for b, g in pairs:
        if True:
            tile_in = inp.tile([P, TG, H], F32)
            src = enc[g * TG * P : (g + 1) * TG * P, b : b + 1, :]
            src = src.rearrange("(t p) o h -> p t (o h)", p=P)
            # alternate the issuing engine: sync and scalar HWDGE rings run
            # in parallel; one ring alone tops out ~20 GB/s below HBM rate
            if (g * BS + b) % 2 == 1 if order == "g" else g % 2 == 1:
                nc.scalar.dma_start(tile_in[:], src)
            else:
                nc.sync.dma_start(tile_in[:], src)
            for t in range(TG):
                c = b * NT + g * TG + t
                nc.vector.affine_mul_reduce(
                    out=dummy.broadcast_to((P, H)),
                    accum_out=en[:, c : c + 1],
                    in0=tile_in[:, t, :],
                    in1=hidb[:, b * H : (b + 1) * H],
                    scale=1.0,
                    bias=0.0,
                )


def softmax_out(nc, work, psum, en, gmat_sb, iden, ones, out_ext):
    # softmax over l (per batch); en[:, c] holds e(l = t*128 + p), c = b*32+t
    m1 = work.tile([P, 1], F32)
    nc.vector.tensor_reduce(
        out=m1[:], in_=en[:], axis=mybir.AxisListType.X, op=mybir.AluOpType.max
    )
    mx = work.tile([P, 1], F32)
    nc.gpsimd.partition_all_reduce(mx[:], m1[:], P, ReduceOp.max)
    negm = work.tile([P, 1], F32)
    nc.scalar.mul(negm[:], mx[:], -1.0)

    p_all = work.tile([P, NC_COLS], F32)
    nc.scalar.activation(
        p_all[:],
        en[:],
        mybir.ActivationFunctionType.Exp,
        bias=negm[:],
        scale=1.0,
    )

    s3 = work.tile([P, BS], F32)
    nc.vector.tensor_reduce(
        out=s3[:],
        in_=p_all[:].rearrange("p (b t) -> p b t", b=BS),
        axis=mybir.AxisListType.X,
        op=mybir.AluOpType.add,
    )
    s_ps = psum.tile([BS, 1], F32)
    nc.tensor.matmul(s_ps[:], s3[:], ones[:], start=True, stop=True)
    r_sb = work.tile([BS, 1], F32)
    nc.vector.reciprocal(r_sb[:], s_ps[:])

    rb_ps = psum.tile([P, 1], F32)
    nc.tensor.matmul(rb_ps[:], gmat_sb[:], r_sb[:], start=True, stop=True)
    rb_sb = work.tile([P, 1], F32)
    nc.scalar.copy(rb_sb[:], rb_ps[:])

    t_ps = psum.tile([P, P], F32)
    nc.tensor.transpose(t_ps[:], p_all[:], iden[:])
    attn_sb = work.tile([P, P], F32)
    nc.vector.tensor_scalar(
        out=attn_sb[:],
        in0=t_ps[:],
        scalar1=rb_sb[:],
        scalar2=None,
        op0=mybir.AluOpType.mult,
    )
    nc.sync.dma_start(out_ext[:], attn_sb[:])


def build_nc(repeat=1, use_for_i=False, order="b"):
    nc = bacc.Bacc(trn_type="TRN2")

    enc = nc.declare_dram_parameter("enc", [L, BS, H], F32, isOutput=False)
    hid = nc.declare_dram_parameter("hid", [1, BS * H], F32, isOutput=False)
    gmat = nc.declare_dram_parameter("gmat", [BS, P], F32, isOutput=False)
    out_ext = nc.declare_dram_parameter("out", [NC_COLS, P], F32, isOutput=True)

    with tile.TileContext(nc) as tc:
        with (
            tc.tile_pool(name="consts", bufs=1) as consts,
            tc.tile_pool(name="inp", bufs=8) as inp,
            tc.tile_pool(name="work", bufs=1) as work,
            tc.tile_pool(name="psum", bufs=1, space="PSUM") as psum,
        ):
            hid_row = consts.tile([1, BS * H], F32)
            nc.sync.dma_start(hid_row[:], hid[:])
            gmat_sb = consts.tile([BS, P], F32)
            nc.sync.dma_start(gmat_sb[:], gmat[:])
            iden = consts.tile([P, P], F32)
            make_identity(nc, iden[:])
            ones = consts.tile([P, 1], F32)
            nc.gpsimd.memset(ones[:], 1.0)

            hidb = consts.tile([P, BS * H], F32)
            nc.gpsimd.partition_broadcast(hidb[:], hid_row[:], P)

            en = work.tile([P, NC_COLS], F32)
            dummy = work.tile([P, 1], F32)
            if repeat == 0:
                nc.gpsimd.memset(en[:], 0.0)

            if use_for_i and repeat > 1:
                with tc.For_i(0, repeat, 1):
                    main_loop(nc, inp, enc, hidb, en, dummy, order)
            else:
                for _rep in range(repeat):
                    main_loop(nc, inp, enc, hidb, en, dummy, order)

            softmax_out(nc, work, psum, en, gmat_sb, iden, ones, out_ext)

    nc.compile()
    return nc


def make_in_maps(hidden, encoder_outputs):
    hidden = np.ascontiguousarray(np.asarray(hidden, dtype=np.float32))
    enc = np.ascontiguousarray(np.asarray(encoder_outputs, dtype=np.float32))
    assert hidden.shape == (H, B) and enc.shape == (L, B, H)

    gmat = np.zeros((BS, P), np.float32)
    for b in range(BS):
        gmat[b, b * NT : (b + 1) * NT] = 1.0

    in_maps = []
    for c in range(NCORES):
        bsl = slice(c * BS, (c + 1) * BS)
        in_maps.append(
            {
                "enc": np.ascontiguousarray(enc[:, bsl, :]),
                "hid": np.ascontiguousarray(hidden[:, bsl].T.reshape(1, BS * H)),
                "gmat": gmat,
            }
        )
    return in_maps


def _get_nc():
    if "nc" not in _cached:
        _cached["nc"] = build_nc()
    return _cached["nc"]


def kernel(hidden, encoder_outputs, **kwargs):
    in_maps = make_in_maps(hidden, encoder_outputs)
    nc = _get_nc()
    res = run_bass_kernel_spmd(nc, in_maps, core_ids=list(range(NCORES)))
    outs = [res.results[i]["out"].reshape(BS, 1, L) for i in range(NCORES)]
    return np.concatenate(outs, axis=0)



# revision 4
# speedup vs baseline: 1.9406x; 1.9406x over previous
"""Bass kernel for nn_Attn_1898375545663 on 8 TRN2 NeuronCores.

Reference (single device):
    energies[b, l] = sum_h hidden[h, b] * encoder_outputs[l, b, h]   # [B, L]
    attn = softmax(energies, axis=1)                                 # [B, L]
    return attn[:, None, :]                                          # [B, 1, L]

Shapes: L=4096, B=32, H=1024. encoder_outputs is 512 MB fp32 -> memory bound.

Sharding: pure data parallel over batch. Each of the 8 cores gets 4 batches;
no collectives.

Key optimization vs the fp32 baseline (200 us): the 2e-2 rel-err budget
admits fp16 inputs (measured l2 rel err 1.9e-3), halving HBM traffic to
32 MB/core -> ~94 us DMA floor. The custom-DVE reduce has no 16-bit fast
path (would be DVE-bound at ~137 us), so the dot products move to the PE:

  - Host pre-transposes each core's shard to [b, hc, h=128, l=4096] fp16 so
    every DMA is a fully contiguous 1 MB block ([128 partitions x 8 KB]).
  - Per (b, hc) tile: 32 matmuls with stationary lhsT = enc[:, lt*128:+128]
    ([K=128 h, M=128 l]) and moving rhs = hid[:, hc*4:+4] ([128 h, 4 b]),
    PSUM-accumulated over hc into mm[128 l, 32*4]. fp16 stationary streams
    at 1 col/cycle @ 2.4 GHz -> ~55 us PE busy, under the DMA floor.
  - Column b of each [128, 4] group is the real dot product (the other 3
    are cross-batch garbage); a strided scalar.copy drops it into
    en[128, c=b*32+lt] -- the same layout the fp32 baseline used, so the
    softmax tail (global max, exp, PE row-sums, reciprocal, PE transpose,
    scale, one output DMA) is unchanged.
"""

import numpy as np

from concourse import bacc, mybir, tile
from concourse.bass_isa import ReduceOp
from concourse.bass_utils import run_bass_kernel_spmd
from concourse.masks import make_identity

L, B, H = 4096, 32, 1024
NCORES = 8
BS = B // NCORES          # 4 batches per core
P = 128                   # partitions
HC = H // P               # 8 h-chunks per batch
NT = L // P               # 32 l-tiles per batch
NC_COLS = BS * NT         # 128 energy columns per core
F32 = mybir.dt.float32
F16 = mybir.dt.float16

_cached = {}


def main_loop(nc, inp, enc, hid_sb, en, psum, z16, tg=2):
    """Stream enc tiles; PE-accumulate energies; drop into en[P, NC_COLS].

    PSUM start=True marks the whole 2 KB bank pending-zero (writes to
    pending bytes store-and-clear, others accumulate), so each batch opens
    its bank with ONE full-width zeroing matmul; everything after
    accumulates with start=False. The opening matmul writes all NT*BS
    columns, which also gives every later matmul a WAW dep on it (keeps
    the scheduler from hoisting an accumulate above the bank reset).
    """
    ngrp = HC // tg  # DMA groups per batch
    for b in range(BS):
        # full bank so rotated buffers never share a zero region
        mm = psum.tile([P, 512], F32)
        nc.tensor.matmul(
            mm[:, : NT * BS], z16[:], z16[:], start=True, stop=False
        )
        for g in range(ngrp):
            tile_in = inp.tile([P, tg, L], F16)
            r0 = (b * HC + g * tg) * P
            src = enc[r0 : r0 + tg * P, :].rearrange("(t p) l -> p t l", p=P)
            if (b * ngrp + g) % 2 == 1:
                nc.scalar.dma_start(tile_in[:], src)
            else:
                nc.sync.dma_start(tile_in[:], src)
            for t in range(tg):
                hc = g * tg + t
                for lt in range(NT):
                    nc.tensor.matmul(
                        mm[:, lt * BS : (lt + 1) * BS],
                        tile_in[:, t, lt * P : (lt + 1) * P],
                        hid_sb[:, hc * BS : (hc + 1) * BS],
                        start=False,
                        stop=(hc == HC - 1 and lt == NT - 1),
                    )
        # column b of each [P, BS] group is this batch's energies
        nc.scalar.copy(
            en[:, b * NT : (b + 1) * NT],
            mm[:, : NT * BS].rearrange("p (lt four) -> p lt four", four=BS)[
                :, :, b
            ],
        )


def softmax_out(nc, work, psum, en, gmat_sb, iden, ones, out_ext):
    # softmax over l (per batch); en[:, c] holds e(l = t*128 + p), c = b*32+t
    m1 = work.tile([P, 1], F32)
    nc.vector.tensor_reduce(
        out=m1[:], in_=en[:], axis=mybir.AxisListType.X, op=mybir.AluOpType.max
    )
    mx = work.tile([P, 1], F32)
    nc.gpsimd.partition_all_reduce(mx[:], m1[:], P, ReduceOp.max)
    negm = work.tile([P, 1], F32)
    nc.scalar.mul(negm[:], mx[:], -1.0)

    p_all = work.tile([P, NC_COLS], F32)
    nc.scalar.activation(
        p_all[:],
        en[:],
        mybir.ActivationFunctionType.Exp,
        bias=negm[:],
        scale=1.0,
    )

    s3 = work.tile([P, BS], F32)
    nc.vector.tensor_reduce(
        out=s3[:],
        in_=p_all[:].rearrange("p (b t) -> p b t", b=BS),
        axis=mybir.AxisListType.X,
        op=mybir.AluOpType.add,
    )
    s_ps = psum.tile([BS, 1], F32)
    nc.tensor.matmul(s_ps[:], s3[:], ones[:], start=True, stop=True)
    r_sb = work.tile([BS, 1], F32)
    nc.vector.reciprocal(r_sb[:], s_ps[:])

    rb_ps = psum.tile([P, 1], F32)
    nc.tensor.matmul(rb_ps[:], gmat_sb[:], r_sb[:], start=True, stop=True)
    rb_sb = work.tile([P, 1], F32)
    nc.scalar.copy(rb_sb[:], rb_ps[:])

    t_ps = psum.tile([P, P], F32)
    nc.tensor.transpose(t_ps[:], p_all[:], iden[:])
    attn_sb = work.tile([P, P], F32)
    nc.vector.tensor_scalar(
        out=attn_sb[:],
        in0=t_ps[:],
        scalar1=rb_sb[:],
        scalar2=None,
        op0=mybir.AluOpType.mult,
    )
    nc.sync.dma_start(out_ext[:], attn_sb[:])


def build_nc(repeat=1, use_for_i=False, tg=2, inp_bufs=6):
    nc = bacc.Bacc(trn_type="TRN2")

    enc = nc.declare_dram_parameter("enc", [BS * H, L], F16, isOutput=False)
    hid = nc.declare_dram_parameter("hid", [P, HC * BS], F16, isOutput=False)
    gmat = nc.declare_dram_parameter("gmat", [BS, P], F32, isOutput=False)
    out_ext = nc.declare_dram_parameter("out", [NC_COLS, P], F32, isOutput=True)

    with tile.TileContext(nc) as tc:
        with (
            tc.tile_pool(name="consts", bufs=1) as consts,
            tc.tile_pool(name="inp", bufs=inp_bufs) as inp,
            tc.tile_pool(name="work", bufs=1) as work,
            tc.tile_pool(name="mmps", bufs=2, space="PSUM") as mmps,
            tc.tile_pool(name="psum", bufs=1, space="PSUM") as psum,
        ):
            hid_sb = consts.tile([P, HC * BS], F16)
            nc.sync.dma_start(hid_sb[:], hid[:])
            gmat_sb = consts.tile([BS, P], F32)
            nc.sync.dma_start(gmat_sb[:], gmat[:])
            iden = consts.tile([P, P], F32)
            make_identity(nc, iden[:])
            ones = consts.tile([P, 1], F32)
            nc.gpsimd.memset(ones[:], 1.0)
            z16 = consts.tile([P, NT * BS], F16)
            nc.gpsimd.memset(z16[:], 0.0)

            en = work.tile([P, NC_COLS], F32)

            if use_for_i and repeat > 1:
                with tc.For_i(0, repeat, 1):
                    main_loop(nc, inp, enc, hid_sb, en, mmps, z16, tg=tg)
            else:
                for _rep in range(repeat):
                    main_loop(nc, inp, enc, hid_sb, en, mmps, z16, tg=tg)

            softmax_out(nc, work, psum, en, gmat_sb, iden, ones, out_ext)

    nc.compile()
    return nc


def make_in_maps(hidden, encoder_outputs):
    hidden = np.asarray(hidden)
    enc = np.asarray(encoder_outputs)
    assert hidden.shape == (H, B) and enc.shape == (L, B, H)

    gmat = np.zeros((BS, P), np.float32)
    for b in range(BS):
        gmat[b, b * NT : (b + 1) * NT] = 1.0

    enc16 = enc.astype(np.float16)
    hid16 = hidden.astype(np.float16)

    in_maps = []
    for c in range(NCORES):
        bsl = slice(c * BS, (c + 1) * BS)
        # [L, BS, H] -> [BS, H, L] -> rows (b, hc, ph), cols l
        enc_t = np.ascontiguousarray(enc16[:, bsl, :].transpose(1, 2, 0)).reshape(
            BS * H, L
        )
        # [H, BS] -> [hc, ph, b] -> [ph, hc*BS + b]
        hid_t = np.ascontiguousarray(
            hid16[:, bsl].reshape(HC, P, BS).transpose(1, 0, 2)
        ).reshape(P, HC * BS)
        in_maps.append({"enc": enc_t, "hid": hid_t, "gmat": gmat})
    return in_maps


def _get_nc():
    if "nc" not in _cached:
        _cached["nc"] = build_nc()
    return _cached["nc"]


def kernel(hidden, encoder_outputs, **kwargs):
    in_maps = make_in_maps(hidden, encoder_outputs)
    nc = _get_nc()
    res = run_bass_kernel_spmd(nc, in_maps, core_ids=list(range(NCORES)))
    outs = [res.results[i]["out"].reshape(BS, 1, L) for i in range(NCORES)]
    return np.concatenate(outs, axis=0)
